# revision 1
# baseline (speedup 1.0000x reference)
"""Trainium2 Bass kernel for CrossModalAttention (MHA + residual + LayerNorm).

Problem: B=4, L=2048, D=256, H=8, Dh=32, fp32.
Sharding: 8 cores; core c handles batch b=c//2, query rows (c%2)*1024..+1024.
Each core computes K/V projections for its full batch (L=2048) - no
cross-core communication needed; host gathers by concatenation.

Per-core dataflow (all layouts chosen to avoid on-device transposes):
  inputs (host-prepped): qT [256,1024], kT [256,2048], vT [256,2048]
  (channel-major), q_res [1024,256] (token-major, for the residual),
  pre-transposed weights WqT/WkT/WvT/WoT [256,256] (= W.T, so contraction
  dim d is on partitions), biases, ln params.

  QT = WqT.T @ qT   [256,1024]  (channel-major - ready to be scores operand)
  KT = WkT.T @ kT   [256,2048]
  V  = vT.T @ WvT   [2048,256]  (token-major), stored interleaved with a
       ones-block per head: vaug[:, 64h:64h+32]=V_h, [.., 64h+32:64h+64]=1
  scoresT_h [k_j, q_i] = KT_h.T @ QT_h   (K=32 contraction, row-strip packed
       2 heads/pass into one 2-bank PSUM tile)
  expS = Exp(scoresT * 1/sqrt(32))       (ScalarE, PSUM->SBUF, FD=1024)
  PV:  [ctx_h; denom_h].T accumulated over k-tiles:
       psum[64e:64e+64] = vaug_h.T @ expS_h   (col-strip packed 2 heads)
       rows 0-31 = ctxT_h (unnormalized), rows 32-63 = softmax denominator
       (replicated 32x by the ones block)
  ctxTn_h = ctx_h / denom_h  (elementwise [32,512] divide, no broadcast)
  out = ctxTn.T @ WoT + bo + q_res ; LayerNorm -> [1024,256]
"""

import numpy as np

import concourse.bass as bass
import concourse.tile as tile
from concourse import bacc, mybir
from concourse.bass_utils import run_bass_kernel_spmd

F32 = mybir.dt.float32
D = 256
H = 8
DH = 32
LQ = 1024  # query rows per core
LK = 2048  # key/value rows per core
P = 128
SCALE = 1.0 / float(np.sqrt(DH))
LN_EPS = 1e-5

N_JT = LK // P  # 16 k-token tiles
N_QC = LQ // 512  # 2 q chunks of 512
N_QT = LQ // P  # 8 q token tiles


def build_nc():
    nc = bacc.Bacc(None)

    qT_d = nc.declare_dram_parameter("qT", [D, LQ], F32, isOutput=False)
    kT_d = nc.declare_dram_parameter("kT", [D, LK], F32, isOutput=False)
    vT_d = nc.declare_dram_parameter("vT", [D, LK], F32, isOutput=False)
    qres_d = nc.declare_dram_parameter("q_res", [LQ, D], F32, isOutput=False)
    wq_d = nc.declare_dram_parameter("WqT", [D, D], F32, isOutput=False)
    wk_d = nc.declare_dram_parameter("WkT", [D, D], F32, isOutput=False)
    wv_d = nc.declare_dram_parameter("WvT", [D, D], F32, isOutput=False)
    wo_d = nc.declare_dram_parameter("WoT", [D, D], F32, isOutput=False)
    biasv_d = nc.declare_dram_parameter("biasv", [4, D], F32, isOutput=False)
    lng_d = nc.declare_dram_parameter("ln_g", [D], F32, isOutput=False)
    lnb_d = nc.declare_dram_parameter("ln_b", [D], F32, isOutput=False)
    out_d = nc.declare_dram_parameter("out", [LQ, D], F32, isOutput=True)

    with tile.TileContext(nc) as tc:
        with (
            tc.tile_pool(name="singles", bufs=1) as singles,
            tc.tile_pool(name="temps", bufs=3) as temps,
            tc.tile_pool(name="mmps", bufs=2, space="PSUM") as mmps,
            tc.tile_pool(name="sps", bufs=2, space="PSUM") as sps,
            tc.tile_pool(name="pvps", bufs=1, space="PSUM") as pvps,
        ):
            # ---- constants / weights -------------------------------------
            wq_sb = singles.tile([P, 2, D], F32, tag="wq")
            wk_sb = singles.tile([P, 2, D], F32, tag="wk")
            wv_sb = singles.tile([P, 2, D], F32, tag="wv")
            wo_sb = singles.tile([P, 2, D], F32, tag="wo")
            for sb, d in ((wq_sb, wq_d), (wk_sb, wk_d), (wv_sb, wv_d), (wo_sb, wo_d)):
                nc.sync.dma_start(out=sb, in_=d.rearrange("(t p) j -> p t j", p=P))

            bias_sb = singles.tile([1, 4, D], F32, tag="biases")
            nc.sync.dma_start(out=bias_sb, in_=biasv_d[None, :, :])
            bq_sb = bias_sb[:, 0, :]
            bk_sb = bias_sb[:, 1, :]
            bv_sb = bias_sb[:, 2, :]
            bo_sb = bias_sb[:, 3, :]

            ones_sb = singles.tile([1, 512], F32, tag="ones")
            nc.vector.memset(ones_sb, 1.0)
            eps_sb = singles.tile([P, 1], F32, tag="eps")
            nc.vector.memset(eps_sb, LN_EPS)

            lng_sb = singles.tile([P, D], F32, tag="lng")
            lnb_sb = singles.tile([P, D], F32, tag="lnb")
            nc.gpsimd.dma_start(out=lng_sb, in_=lng_d[None, :].to_broadcast((P, D)))
            nc.gpsimd.dma_start(out=lnb_sb, in_=lnb_d[None, :].to_broadcast((P, D)))

            # ---- activation inputs (channel-major) -----------------------
            xq_sb = singles.tile([P, 2, LQ], F32, tag="xq")
            xk_sb = singles.tile([P, 2, LK], F32, tag="xk")
            xv_sb = singles.tile([P, 2, LK], F32, tag="xv")
            nc.sync.dma_start(out=xq_sb, in_=qT_d.rearrange("(t p) l -> p t l", p=P))
            nc.sync.dma_start(out=xk_sb, in_=kT_d.rearrange("(t p) l -> p t l", p=P))
            nc.sync.dma_start(out=xv_sb, in_=vT_d.rearrange("(t p) l -> p t l", p=P))
            qres_sb = singles.tile([P, N_QT, D], F32, tag="qres")
            nc.sync.dma_start(
                out=qres_sb, in_=qres_d.rearrange("(t p) d -> p t d", p=P)
            )

            # ---- persistent activations ----------------------------------
            QT_sb = singles.tile([P, 2, LQ], F32, tag="QT")
            KT_sb = singles.tile([P, 2, LK], F32, tag="KT")
            vaug = [
                singles.tile([P, H * 64], F32, tag=f"vaug{t}", name=f"vaug{t}")
                for t in range(N_JT)
            ]
            ctxTn = singles.tile([P, 2, LQ], F32, tag="ctxTn")
            y_sb = singles.tile([P, N_QT, D], F32, tag="y")
            mv_sb = singles.tile([P, N_QT, 2], F32, tag="mv")
            sd_sb = singles.tile([P, N_QT], F32, tag="sd")
            rstd_sb = singles.tile([P, N_QT], F32, tag="rstd")

            # ---- phase A: QKV projections --------------------------------
            # QT[j, t] = sum_d WqT[d, j] * qT[d, t] + bq[j]
            for jt in range(2):
                for qcc in range(2):
                    ps = mmps.tile([P, 512], F32, tag="mm")
                    nc.tensor.matmul(
                        ps,
                        lhsT=wq_sb[:, 0, jt * P : (jt + 1) * P],
                        rhs=xq_sb[:, 0, qcc * 512 : (qcc + 1) * 512],
                        start=True,
                        stop=False,
                    )
                    nc.tensor.matmul(
                        ps,
                        lhsT=wq_sb[:, 1, jt * P : (jt + 1) * P],
                        rhs=xq_sb[:, 1, qcc * 512 : (qcc + 1) * 512],
                        start=False,
                        stop=False,
                    )
                    nc.tensor.matmul(
                        ps,
                        lhsT=bq_sb[:, jt * P : (jt + 1) * P],
                        rhs=ones_sb[:, :512],
                        start=False,
                        stop=True,
                    )
                    nc.vector.tensor_copy(
                        out=QT_sb[:, jt, qcc * 512 : (qcc + 1) * 512], in_=ps
                    )
            for jt in range(2):
                for kc in range(4):
                    ps = mmps.tile([P, 512], F32, tag="mm")
                    nc.tensor.matmul(
                        ps,
                        lhsT=wk_sb[:, 0, jt * P : (jt + 1) * P],
                        rhs=xk_sb[:, 0, kc * 512 : (kc + 1) * 512],
                        start=True,
                        stop=False,
                    )
                    nc.tensor.matmul(
                        ps,
                        lhsT=wk_sb[:, 1, jt * P : (jt + 1) * P],
                        rhs=xk_sb[:, 1, kc * 512 : (kc + 1) * 512],
                        start=False,
                        stop=False,
                    )
                    nc.tensor.matmul(
                        ps,
                        lhsT=bk_sb[:, jt * P : (jt + 1) * P],
                        rhs=ones_sb[:, :512],
                        start=False,
                        stop=True,
                    )
                    nc.vector.tensor_copy(
                        out=KT_sb[:, jt, kc * 512 : (kc + 1) * 512], in_=ps
                    )
            # V token-major, written interleaved into vaug with ones blocks
            for tt in range(N_JT):
                ps = mmps.tile([P, D], F32, tag="mm")
                nc.tensor.matmul(
                    ps,
                    lhsT=xv_sb[:, 0, tt * P : (tt + 1) * P],
                    rhs=wv_sb[:, 0, :],
                    start=True,
                    stop=False,
                )
                nc.tensor.matmul(
                    ps,
                    lhsT=xv_sb[:, 1, tt * P : (tt + 1) * P],
                    rhs=wv_sb[:, 1, :],
                    start=False,
                    stop=False,
                )
                nc.tensor.matmul(
                    ps,
                    lhsT=ones_sb[:1, :P],
                    rhs=bv_sb,
                    start=False,
                    stop=True,
                )
                vt = vaug[tt].rearrange("p (h c) -> p h c", c=64)
                nc.vector.memset(vt[:, :, DH:], 1.0)
                nc.vector.tensor_copy(
                    out=vt[:, :, :DH],
                    in_=ps.rearrange("p (h c) -> p h c", c=DH),
                )

            # ---- attention ----------------------------------------------
            for qc in range(N_QC):
                q0 = qc * 512
                cu = temps.tile([P, 2, 512], F32, tag="cu")  # unnormalized ctxT
                den = temps.tile([P, 2, 512], F32, tag="den")  # denominators
                for hp in range(4):  # head pairs (2hp, 2hp+1)
                    pv = pvps.tile([P, 2, 512], F32, tag="pv")
                    for jt in range(N_JT):
                        s = sps.tile([P, 2, 512], F32, tag="s")
                        for e in range(2):
                            h = 2 * hp + e
                            dt = h // 4
                            r0 = (h % 4) * DH
                            nc.tensor.matmul(
                                s[:, e, :],
                                lhsT=KT_sb[r0 : r0 + DH, dt, jt * P : (jt + 1) * P],
                                rhs=QT_sb[r0 : r0 + DH, dt, q0 : q0 + 512],
                                start=True,
                                stop=True,
                                tile_position=(r0, 0),
                            )
                        es = temps.tile([P, 2, 512], F32, tag="es")
                        nc.scalar.activation(
                            out=es,
                            in_=s,
                            func=mybir.ActivationFunctionType.Exp,
                            scale=SCALE,
                        )
                        for e in range(2):
                            h = 2 * hp + e
                            # each head accumulates in its own PSUM bank
                            # (col-strip packing miscomputes on this stack)
                            nc.tensor.matmul(
                                pv[0:64, e, :],
                                lhsT=vaug[jt][:, 64 * h : 64 * h + 64],
                                rhs=es[:, e, :],
                                start=(jt == 0),
                                stop=(jt == N_JT - 1),
                            )
                    # stage ctx + denominator rows into SBUF at the ctxTn row
                    # layout (rows 32*(h%4) of partition-tile h//4); the
                    # reciprocal runs batched from SBUF afterwards (reciprocal
                    # with a PSUM source miscomputes/crashes on this stack)
                    for e in range(2):
                        h = 2 * hp + e
                        dt = h // 4
                        r0 = (h % 4) * DH
                        nc.vector.tensor_copy(
                            out=cu[r0 : r0 + DH, dt, :], in_=pv[0:DH, e, :]
                        )
                        nc.vector.tensor_copy(
                            out=den[r0 : r0 + DH, dt, :], in_=pv[DH:64, e, :]
                        )
                # normalize all 8 heads for this q chunk: 2 reciprocals + 2 mults
                rec = temps.tile([P, 2, 512], F32, tag="rec")
                nc.vector.reciprocal(out=rec, in_=den)
                for dtv in range(2):
                    nc.vector.tensor_tensor(
                        out=ctxTn[:, dtv, q0 : q0 + 512],
                        in0=cu[:, dtv, :],
                        in1=rec[:, dtv, :],
                        op=mybir.AluOpType.mult,
                    )

                # ---- output projection + residual for this q chunk -------
                for q4 in range(4):
                    qt = qc * 4 + q4
                    po = mmps.tile([P, D], F32, tag="mm")
                    nc.tensor.matmul(
                        po,
                        lhsT=ctxTn[:, 0, qt * P : (qt + 1) * P],
                        rhs=wo_sb[:, 0, :],
                        start=True,
                        stop=False,
                    )
                    nc.tensor.matmul(
                        po,
                        lhsT=ctxTn[:, 1, qt * P : (qt + 1) * P],
                        rhs=wo_sb[:, 1, :],
                        start=False,
                        stop=False,
                    )
                    nc.tensor.matmul(
                        po,
                        lhsT=ones_sb[:1, :P],
                        rhs=bo_sb,
                        start=False,
                        stop=True,
                    )
                    nc.vector.tensor_add(out=y_sb[:, qt, :], in0=po, in1=qres_sb[:, qt, :])
                    st = temps.tile([P, 6], F32, tag="st")
                    nc.vector.bn_stats(out=st, in_=y_sb[:, qt, :])
                    nc.vector.bn_aggr(out=mv_sb[:, qt, :], in_=st)

            # ---- final LayerNorm pass (one ACT table switch) -------------
            nc.scalar.activation(
                out=sd_sb,
                in_=mv_sb[:, :, 1:2],
                func=mybir.ActivationFunctionType.Sqrt,
                bias=eps_sb,
            )
            nc.vector.reciprocal(out=rstd_sb, in_=sd_sb)
            for qt in range(N_QT):
                nc.vector.tensor_scalar(
                    out=y_sb[:, qt, :],
                    in0=y_sb[:, qt, :],
                    scalar1=mv_sb[:, qt, 0:1],
                    scalar2=rstd_sb[:, qt : qt + 1],
                    op0=mybir.AluOpType.subtract,
                    op1=mybir.AluOpType.mult,
                )
                nc.vector.tensor_tensor(
                    out=y_sb[:, qt, :],
                    in0=y_sb[:, qt, :],
                    in1=lng_sb,
                    op=mybir.AluOpType.mult,
                )
                nc.vector.tensor_add(out=y_sb[:, qt, :], in0=y_sb[:, qt, :], in1=lnb_sb)
            nc.sync.dma_start(
                out=out_d.rearrange("(t p) d -> p t d", p=P), in_=y_sb
            )

    nc.finalize()
    return nc


_NC_CACHE = None


def _get_nc():
    global _NC_CACHE
    if _NC_CACHE is None:
        _NC_CACHE = build_nc()
    return _NC_CACHE


def make_in_maps(query, key, value, Wq, bq, Wk, bk, Wv, bv, Wo, bo, ln_g, ln_b):
    f = lambda x: np.ascontiguousarray(np.asarray(x, dtype=np.float32))
    shared = {
        "WqT": f(np.asarray(Wq).T),
        "WkT": f(np.asarray(Wk).T),
        "WvT": f(np.asarray(Wv).T),
        "WoT": f(np.asarray(Wo).T),
        "biasv": f(np.stack([np.asarray(bq), np.asarray(bk), np.asarray(bv), np.asarray(bo)])),
        "ln_g": f(ln_g),
        "ln_b": f(ln_b),
    }
    query = np.asarray(query, dtype=np.float32)
    key = np.asarray(key, dtype=np.float32)
    value = np.asarray(value, dtype=np.float32)
    in_maps = []
    for c in range(8):
        b, half = c // 2, c % 2
        lo = half * LQ
        in_maps.append(
            dict(
                shared,
                qT=f(query[b, lo : lo + LQ, :].T),
                kT=f(key[b].T),
                vT=f(value[b].T),
                q_res=f(query[b, lo : lo + LQ, :]),
            )
        )
    return in_maps


def kernel(query, key, value, Wq, bq, Wk, bk, Wv, bv, Wo, bo, ln_g, ln_b):
    nc = _get_nc()
    in_maps = make_in_maps(
        query, key, value, Wq, bq, Wk, bk, Wv, bv, Wo, bo, ln_g, ln_b
    )
    res = run_bass_kernel_spmd(nc, in_maps, core_ids=list(range(8)))
    out = np.empty((4, 2048, 256), dtype=np.float32)
    for c in range(8):
        b, half = c // 2, c % 2
        out[b, half * LQ : (half + 1) * LQ, :] = res.results[c]["out"]
    return out



# revision 6
# speedup vs baseline: 4.3058x; 4.3058x over previous
"""Trainium2 Bass kernel for CrossModalAttention (MHA + residual + LayerNorm).

Problem: B=4, L=2048, D=256, H=8, Dh=32, fp32 reference.
Sharding: 8 cores; core c handles batch b=c//2, query rows (c%2)*1024..+1024.
Each core computes K/V projections for its full batch (L=2048) - no
cross-core communication needed; host gathers by concatenation.

Dispatch-cost note: in this axon-tunneled environment the per-execution
cost is dominated by the NUMBER of NEFF I/O buffers (~2ms per operand
per dispatch) with bytes comparatively cheap. So all inputs are packed
into ONE bf16 blob [128, CBLOB] laid out in device tile order (single
contiguous DMA on device), and the output is ONE bf16 tensor. bf16 also
halves bytes and doubles TensorE throughput; fp32 is kept for PSUM
accumulation, softmax denominators/reciprocals and LayerNorm stats.

Blob column layout (all bf16, partition dim 128 first):
  XQ  [128, 2, 1024]  qT channel-major  (cols     0.. 2048)
  XK  [128, 2, 2048]  kT channel-major  (cols  2048.. 6144)
  XV  [128, 2, 2048]  vT channel-major  (cols  6144..10240)
  WQ/WK/WV/WO [128, 2, 256] each = W.T in (t p) j -> p t j (10240..12288)
  SMALL [128, 6, 256] rows replicated: bq,bk,bv,bo,ln_g,ln_b (12288..13824)
  IDN [128, 128] identity for TensorE transposes          (13824..13952)

Per-core dataflow (layouts avoid all host-side transposes of outputs):
  QT = WqT.T @ qT   [256,1024]  (channel-major - ready to be scores operand)
  KT = WkT.T @ kT   [256,2048]
  V  = vT.T @ WvT   [2048,256]  (token-major), stored interleaved with a
       ones-block per head: vaug[:, 64h:64h+32]=V_h, [.., 64h+32:64h+64]=1
  q_res (token-major residual) = TensorE transpose of XQ tiles
  scoresT_h [k_j, q_i] = KT_h.T @ QT_h   (K=32 contraction, row-strip packed
       2 heads/pass into one 2-bank PSUM tile)
  expS = Exp(scoresT * 1/sqrt(32))       (ScalarE, PSUM->SBUF bf16, FD=1024)
  PV:  [ctx_h; denom_h].T accumulated over k-tiles:
       psum[0:64] = vaug_h.T @ expS_h
       rows 0-31 = ctxT_h (unnormalized), rows 32-63 = softmax denominator
       (replicated 32x by the ones block)
  ctxTn_h = ctx_h / denom_h  (fp32 cu/den from PSUM; product written bf16)
  out = ctxTn.T @ WoT + bo + q_res ; LayerNorm -> [1024,256] bf16
"""

import ml_dtypes
import numpy as np

import concourse.bass as bass
import concourse.tile as tile
from concourse import bacc, mybir
from concourse.bass_utils import run_bass_kernel_spmd

F32 = mybir.dt.float32
BF16 = mybir.dt.bfloat16
NPBF16 = ml_dtypes.bfloat16
D = 256
H = 8
DH = 32
LQ = 1024  # query rows per core
LK = 2048  # key/value rows per core
P = 128
SCALE = 1.0 / float(np.sqrt(DH))
LN_EPS = 1e-5

N_JT = LK // P  # 16 k-token tiles
N_QC = LQ // 512  # 2 q chunks of 512
N_QT = LQ // P  # 8 q token tiles

# blob column offsets
C_XQ = 0
C_XK = C_XQ + 2 * LQ  # 2048
C_XV = C_XK + 2 * LK  # 6144
C_W = C_XV + 2 * LK  # 10240  (WQ, WK, WV, WO each 2*256)
C_SMALL = C_W + 4 * 2 * D  # 12288  (bq,bk,bv,bo,ln_g,ln_b each 256)
C_IDN = C_SMALL + 6 * D  # 13824
CBLOB = C_IDN + P  # 13952


def build_nc():
    nc = bacc.Bacc(None)

    blob_d = nc.declare_dram_parameter("blob", [P, CBLOB], BF16, isOutput=False)
    out_d = nc.declare_dram_parameter("out", [LQ, D], BF16, isOutput=True)

    with tile.TileContext(nc) as tc:
        with (
            tc.tile_pool(name="singles", bufs=1) as singles,
            tc.tile_pool(name="temps", bufs=3) as temps,
            tc.tile_pool(name="mmps", bufs=2, space="PSUM") as mmps,
            tc.tile_pool(name="sps", bufs=2, space="PSUM") as sps,
            tc.tile_pool(name="pvps", bufs=1, space="PSUM") as pvps,
        ):
            # ---- the one input DMA ---------------------------------------
            blob = singles.tile([P, CBLOB], BF16, tag="blob")
            nc.sync.dma_start(out=blob, in_=blob_d[:, :])

            xq_sb = blob[:, C_XQ : C_XQ + 2 * LQ].rearrange("p (t l) -> p t l", t=2)
            xk_sb = blob[:, C_XK : C_XK + 2 * LK].rearrange("p (t l) -> p t l", t=2)
            xv_sb = blob[:, C_XV : C_XV + 2 * LK].rearrange("p (t l) -> p t l", t=2)
            wq_sb = blob[:, C_W + 0 * 2 * D : C_W + 1 * 2 * D].rearrange(
                "p (t j) -> p t j", t=2
            )
            wk_sb = blob[:, C_W + 1 * 2 * D : C_W + 2 * 2 * D].rearrange(
                "p (t j) -> p t j", t=2
            )
            wv_sb = blob[:, C_W + 2 * 2 * D : C_W + 3 * 2 * D].rearrange(
                "p (t j) -> p t j", t=2
            )
            wo_sb = blob[:, C_W + 3 * 2 * D : C_W + 4 * 2 * D].rearrange(
                "p (t j) -> p t j", t=2
            )
            small = blob[:, C_SMALL : C_SMALL + 6 * D].rearrange(
                "p (k d) -> p k d", k=6
            )
            bq_sb = small[0:1, 0, :]
            bk_sb = small[0:1, 1, :]
            bv_sb = small[0:1, 2, :]
            bo_sb = small[0:1, 3, :]
            lng_sb = small[:, 4, :]
            lnb_sb = small[:, 5, :]
            idn_sb = blob[:, C_IDN : C_IDN + P]

            ones_sb = singles.tile([1, 512], BF16, tag="ones")
            nc.vector.memset(ones_sb, 1.0)
            eps_sb = singles.tile([P, 1], F32, tag="eps")
            nc.vector.memset(eps_sb, LN_EPS)

            # ---- persistent activations ----------------------------------
            QT_sb = singles.tile([P, 2, LQ], BF16, tag="QT")
            KT_sb = singles.tile([P, 2, LK], BF16, tag="KT")
            vaug = [
                singles.tile([P, H * 64], BF16, tag=f"vaug{t}", name=f"vaug{t}")
                for t in range(N_JT)
            ]
            ctxTn = singles.tile([P, 2, LQ], BF16, tag="ctxTn")
            y_sb = singles.tile([P, N_QT, D], F32, tag="y")
            yo_sb = singles.tile([P, N_QT, D], BF16, tag="yo")
            mv_sb = singles.tile([P, N_QT, 2], F32, tag="mv")
            sd_sb = singles.tile([P, N_QT], F32, tag="sd")
            rstd_sb = singles.tile([P, N_QT], F32, tag="rstd")

            # ---- phase A: QKV projections --------------------------------
            # QT[j, t] = sum_d WqT[d, j] * qT[d, t] + bq[j]
            for jt in range(2):
                for qcc in range(2):
                    ps = mmps.tile([P, 512], F32, tag="mm")
                    nc.tensor.matmul(
                        ps,
                        lhsT=wq_sb[:, 0, jt * P : (jt + 1) * P],
                        rhs=xq_sb[:, 0, qcc * 512 : (qcc + 1) * 512],
                        start=True,
                        stop=False,
                    )
                    nc.tensor.matmul(
                        ps,
                        lhsT=wq_sb[:, 1, jt * P : (jt + 1) * P],
                        rhs=xq_sb[:, 1, qcc * 512 : (qcc + 1) * 512],
                        start=False,
                        stop=False,
                    )
                    nc.tensor.matmul(
                        ps,
                        lhsT=bq_sb[:, jt * P : (jt + 1) * P],
                        rhs=ones_sb[:, :512],
                        start=False,
                        stop=True,
                    )
                    nc.vector.tensor_copy(
                        out=QT_sb[:, jt, qcc * 512 : (qcc + 1) * 512], in_=ps
                    )
            for jt in range(2):
                for kc in range(4):
                    ps = mmps.tile([P, 512], F32, tag="mm")
                    nc.tensor.matmul(
                        ps,
                        lhsT=wk_sb[:, 0, jt * P : (jt + 1) * P],
                        rhs=xk_sb[:, 0, kc * 512 : (kc + 1) * 512],
                        start=True,
                        stop=False,
                    )
                    nc.tensor.matmul(
                        ps,
                        lhsT=wk_sb[:, 1, jt * P : (jt + 1) * P],
                        rhs=xk_sb[:, 1, kc * 512 : (kc + 1) * 512],
                        start=False,
                        stop=False,
                    )
                    nc.tensor.matmul(
                        ps,
                        lhsT=bk_sb[:, jt * P : (jt + 1) * P],
                        rhs=ones_sb[:, :512],
                        start=False,
                        stop=True,
                    )
                    nc.vector.tensor_copy(
                        out=KT_sb[:, jt, kc * 512 : (kc + 1) * 512], in_=ps
                    )
            # V token-major, written interleaved into vaug with ones blocks
            for tt in range(N_JT):
                ps = mmps.tile([P, D], F32, tag="mm")
                nc.tensor.matmul(
                    ps,
                    lhsT=xv_sb[:, 0, tt * P : (tt + 1) * P],
                    rhs=wv_sb[:, 0, :],
                    start=True,
                    stop=False,
                )
                nc.tensor.matmul(
                    ps,
                    lhsT=xv_sb[:, 1, tt * P : (tt + 1) * P],
                    rhs=wv_sb[:, 1, :],
                    start=False,
                    stop=False,
                )
                nc.tensor.matmul(
                    ps,
                    lhsT=ones_sb[:1, :P],
                    rhs=bv_sb,
                    start=False,
                    stop=True,
                )
                vt = vaug[tt].rearrange("p (h c) -> p h c", c=64)
                nc.vector.memset(vt[:, :, DH:], 1.0)
                nc.vector.tensor_copy(
                    out=vt[:, :, :DH],
                    in_=ps.rearrange("p (h c) -> p h c", c=DH),
                )

            # ---- attention ----------------------------------------------
            for qc in range(N_QC):
                q0 = qc * 512
                cu = temps.tile([P, 2, 512], F32, tag="cu")  # unnormalized ctxT
                den = temps.tile([P, 2, 512], F32, tag="den")  # denominators
                for hp in range(4):  # head pairs (2hp, 2hp+1)
                    pv = pvps.tile([P, 2, 512], F32, tag="pv")
                    for jt in range(N_JT):
                        s = sps.tile([P, 2, 512], F32, tag="s")
                        for e in range(2):
                            h = 2 * hp + e
                            dt = h // 4
                            r0 = (h % 4) * DH
                            nc.tensor.matmul(
                                s[:, e, :],
                                lhsT=KT_sb[r0 : r0 + DH, dt, jt * P : (jt + 1) * P],
                                rhs=QT_sb[r0 : r0 + DH, dt, q0 : q0 + 512],
                                start=True,
                                stop=True,
                                tile_position=(r0, 0),
                            )
                        es = temps.tile([P, 2, 512], BF16, tag="es")
                        nc.scalar.activation(
                            out=es,
                            in_=s,
                            func=mybir.ActivationFunctionType.Exp,
                            scale=SCALE,
                        )
                        for e in range(2):
                            h = 2 * hp + e
                            # each head accumulates in its own PSUM bank
                            # (col-strip packing miscomputes on this stack)
                            nc.tensor.matmul(
                                pv[0:64, e, :],
                                lhsT=vaug[jt][:, 64 * h : 64 * h + 64],
                                rhs=es[:, e, :],
                                start=(jt == 0),
                                stop=(jt == N_JT - 1),
                            )
                    # stage ctx + denominator rows into SBUF at the ctxTn row
                    # layout (rows 32*(h%4) of partition-tile h//4); the
                    # reciprocal runs batched from SBUF afterwards (reciprocal
                    # with a PSUM source miscomputes/crashes on this stack)
                    for e in range(2):
                        h = 2 * hp + e
                        dt = h // 4
                        r0 = (h % 4) * DH
                        nc.vector.tensor_copy(
                            out=cu[r0 : r0 + DH, dt, :], in_=pv[0:DH, e, :]
                        )
                        nc.vector.tensor_copy(
                            out=den[r0 : r0 + DH, dt, :], in_=pv[DH:64, e, :]
                        )
                # normalize all 8 heads for this q chunk: 2 reciprocals + 2 mults
                rec = temps.tile([P, 2, 512], F32, tag="rec")
                nc.vector.reciprocal(out=rec, in_=den)
                for dtv in range(2):
                    nc.vector.tensor_tensor(
                        out=ctxTn[:, dtv, q0 : q0 + 512],
                        in0=cu[:, dtv, :],
                        in1=rec[:, dtv, :],
                        op=mybir.AluOpType.mult,
                    )

                # ---- output projection + residual for this q chunk -------
                for q4 in range(4):
                    qt = qc * 4 + q4
                    po = mmps.tile([P, D], F32, tag="mm")
                    nc.tensor.matmul(
                        po,
                        lhsT=ctxTn[:, 0, qt * P : (qt + 1) * P],
                        rhs=wo_sb[:, 0, :],
                        start=True,
                        stop=False,
                    )
                    nc.tensor.matmul(
                        po,
                        lhsT=ctxTn[:, 1, qt * P : (qt + 1) * P],
                        rhs=wo_sb[:, 1, :],
                        start=False,
                        stop=False,
                    )
                    nc.tensor.matmul(
                        po,
                        lhsT=ones_sb[:1, :P],
                        rhs=bo_sb,
                        start=False,
                        stop=False,
                    )
                    # residual: accumulate token-major q into the same PSUM
                    # tile via identity matmul: po[t, dt*128+n] += xq[n, t]
                    for dt in range(2):
                        nc.tensor.matmul(
                            po[:, dt * P : (dt + 1) * P],
                            lhsT=xq_sb[:, dt, qt * P : (qt + 1) * P],
                            rhs=idn_sb,
                            start=False,
                            stop=(dt == 1),
                        )
                    nc.vector.tensor_copy(out=y_sb[:, qt, :], in_=po)
                    st = temps.tile([P, 6], F32, tag="st")
                    nc.vector.bn_stats(out=st, in_=y_sb[:, qt, :])
                    nc.vector.bn_aggr(out=mv_sb[:, qt, :], in_=st)

            # ---- final LayerNorm pass (one ACT table switch) -------------
            nc.scalar.activation(
                out=sd_sb,
                in_=mv_sb[:, :, 1:2],
                func=mybir.ActivationFunctionType.Sqrt,
                bias=eps_sb,
            )
            nc.vector.reciprocal(out=rstd_sb, in_=sd_sb)
            for qt in range(N_QT):
                nc.vector.tensor_scalar(
                    out=y_sb[:, qt, :],
                    in0=y_sb[:, qt, :],
                    scalar1=mv_sb[:, qt, 0:1],
                    scalar2=rstd_sb[:, qt : qt + 1],
                    op0=mybir.AluOpType.subtract,
                    op1=mybir.AluOpType.mult,
                )
                nc.vector.tensor_tensor(
                    out=y_sb[:, qt, :],
                    in0=y_sb[:, qt, :],
                    in1=lng_sb,
                    op=mybir.AluOpType.mult,
                )
                nc.vector.tensor_add(
                    out=yo_sb[:, qt, :], in0=y_sb[:, qt, :], in1=lnb_sb
                )
            nc.sync.dma_start(
                out=out_d.rearrange("(t p) d -> p t d", p=P), in_=yo_sb
            )

    nc.finalize()
    return nc


_NC_CACHE = None


def _get_nc():
    global _NC_CACHE
    if _NC_CACHE is None:
        _NC_CACHE = build_nc()
    return _NC_CACHE


def _pack_blob(qT, kT, vT, WqT, WkT, WvT, WoT, smallv):
    """Pack per-core tensors into the [P, CBLOB] bf16 blob (tile layouts)."""
    blob = np.empty((P, CBLOB), dtype=NPBF16)

    def chan_major(x, cols):  # x [256, L] -> [128, 2*L] as (p, t, l)
        L = x.shape[1]
        blob[:, cols : cols + 2 * L] = (
            x.reshape(2, P, L).transpose(1, 0, 2).reshape(P, 2 * L)
        )

    chan_major(qT, C_XQ)
    chan_major(kT, C_XK)
    chan_major(vT, C_XV)
    for i, w in enumerate((WqT, WkT, WvT, WoT)):
        chan_major(w, C_W + i * 2 * D)
    # SMALL: 6 rows of 256, replicated across the 128 partitions
    blob[:, C_SMALL : C_SMALL + 6 * D] = smallv.reshape(1, 6 * D)
    blob[:, C_IDN : C_IDN + P] = np.eye(P, dtype=NPBF16)
    return blob


def make_in_maps(query, key, value, Wq, bq, Wk, bk, Wv, bv, Wo, bo, ln_g, ln_b):
    f16 = lambda x: np.asarray(x, dtype=np.float32).astype(NPBF16)
    WqT, WkT, WvT, WoT = (
        f16(np.asarray(w, dtype=np.float32).T) for w in (Wq, Wk, Wv, Wo)
    )
    smallv = f16(np.stack([bq, bk, bv, bo, ln_g, ln_b]))
    query = np.asarray(query, dtype=np.float32)
    key = np.asarray(key, dtype=np.float32)
    value = np.asarray(value, dtype=np.float32)
    in_maps = []
    for c in range(8):
        b, half = c // 2, c % 2
        lo = half * LQ
        blob = _pack_blob(
            f16(query[b, lo : lo + LQ, :].T),
            f16(key[b].T),
            f16(value[b].T),
            WqT,
            WkT,
            WvT,
            WoT,
            smallv,
        )
        in_maps.append({"blob": blob})
    return in_maps


def kernel(query, key, value, Wq, bq, Wk, bk, Wv, bv, Wo, bo, ln_g, ln_b):
    nc = _get_nc()
    in_maps = make_in_maps(
        query, key, value, Wq, bq, Wk, bk, Wv, bv, Wo, bo, ln_g, ln_b
    )
    res = run_bass_kernel_spmd(nc, in_maps, core_ids=list(range(8)))
    out = np.empty((4, 2048, 256), dtype=np.float32)
    for c in range(8):
        b, half = c // 2, c % 2
        out[b, half * LQ : (half + 1) * LQ, :] = np.asarray(
            res.results[c]["out"], dtype=np.float32
        )
    return out


# revision 7
# speedup vs baseline: 21.7321x; 5.0472x over previous
"""Trainium2 Bass kernel for CrossModalAttention (MHA + residual + LayerNorm).

Problem: B=4, L=2048, D=256, H=8, Dh=32, fp32 reference.
Sharding: 8 cores; core c handles batch b=c//2, query rows (c%2)*1024..+1024.
Each core computes K/V projections for its full batch (L=2048) - no
cross-core communication needed; host gathers by concatenation.

Dispatch-cost note: in this axon-tunneled environment the per-execution
cost is dominated by the NUMBER of NEFF I/O buffers (~2ms per operand
per dispatch) with bytes comparatively cheap. So all inputs are packed
into ONE bf16 blob [128, CBLOB] laid out in device tile order (single
contiguous DMA on device), and the output is ONE bf16 tensor. bf16 also
halves bytes and doubles TensorE throughput; fp32 is kept for PSUM
accumulation, softmax denominators/reciprocals and LayerNorm stats.

Blob column layout (all bf16, partition dim 128 first):
  XQ  [128, 2, 1024]  qT channel-major  (cols     0.. 2048)
  XK  [128, 2, 2048]  kT channel-major  (cols  2048.. 6144)
  XV  [128, 2, 2048]  vT channel-major  (cols  6144..10240)
  WQ/WK/WV/WO [128, 2, 256] each = W.T in (t p) j -> p t j (10240..12288)
  SMALL [128, 6, 256] rows replicated: bq,bk,bv,bo,ln_g,ln_b (12288..13824)
  IDN [128, 128] identity for TensorE transposes          (13824..13952)

Per-core dataflow (layouts avoid all host-side transposes of outputs):
  QT = WqT.T @ qT   [256,1024]  (channel-major - ready to be scores operand)
  KT = WkT.T @ kT   [256,2048]
  V  = vT.T @ WvT   [2048,256]  (token-major), stored interleaved with a
       ones-block per head: vaug[:, 64h:64h+32]=V_h, [.., 64h+32:64h+64]=1
  q_res (token-major residual) = TensorE transpose of XQ tiles
  scoresT_h [k_j, q_i] = KT_h.T @ QT_h   (K=32 contraction, row-strip packed
       2 heads/pass into one 2-bank PSUM tile)
  expS = Exp(scoresT * 1/sqrt(32))       (ScalarE, PSUM->SBUF bf16, FD=1024)
  PV:  [ctx_h; denom_h].T accumulated over k-tiles:
       psum[0:64] = vaug_h.T @ expS_h
       rows 0-31 = ctxT_h (unnormalized), rows 32-63 = softmax denominator
       (replicated 32x by the ones block)
  ctxTn_h = ctx_h / denom_h  (fp32 cu/den from PSUM; product written bf16)
  out = ctxTn.T @ WoT + bo + q_res ; LayerNorm -> [1024,256] bf16
"""

import ml_dtypes
import numpy as np

import concourse.bass as bass
import concourse.tile as tile
from concourse import bacc, mybir
from concourse.bass_utils import run_bass_kernel_spmd

F32 = mybir.dt.float32
BF16 = mybir.dt.bfloat16
NPBF16 = ml_dtypes.bfloat16
D = 256
H = 8
DH = 32
LQ = 1024  # query rows per core
LK = 2048  # key/value rows per core
P = 128
SCALE = 1.0 / float(np.sqrt(DH))
LN_EPS = 1e-5

N_JT = LK // P  # 16 k-token tiles
N_QC = LQ // 512  # 2 q chunks of 512
N_QT = LQ // P  # 8 q token tiles

# blob column offsets
C_XQ = 0
C_XK = C_XQ + 2 * LQ  # 2048
C_XV = C_XK + 2 * LK  # 6144
C_W = C_XV + 2 * LK  # 10240  (WQ, WK, WV, WO each 2*256)
C_SMALL = C_W + 4 * 2 * D  # 12288  (bq,bk,bv,bo,ln_g,ln_b each 256)
C_IDN = C_SMALL + 6 * D  # 13824
CBLOB = C_IDN + P  # 13952


def build_nc():
    # no partition_id input: the kernel is pure SPMD (per-core data is
    # selected host-side), and every NEFF operand costs ~ms per dispatch
    # in the axon-tunneled environment.
    nc = bacc.Bacc(None, enable_partition_id=False)

    blob_d = nc.declare_dram_parameter("blob", [P, CBLOB], BF16, isOutput=False)
    out_d = nc.declare_dram_parameter("out", [LQ, D], BF16, isOutput=True)

    with tile.TileContext(nc) as tc:
        with (
            tc.tile_pool(name="singles", bufs=1) as singles,
            tc.tile_pool(name="temps", bufs=3) as temps,
            tc.tile_pool(name="mmps", bufs=2, space="PSUM") as mmps,
            tc.tile_pool(name="sps", bufs=2, space="PSUM") as sps,
            tc.tile_pool(name="pvps", bufs=1, space="PSUM") as pvps,
        ):
            # ---- the one input DMA ---------------------------------------
            blob = singles.tile([P, CBLOB], BF16, tag="blob")
            nc.sync.dma_start(out=blob, in_=blob_d[:, :])

            xq_sb = blob[:, C_XQ : C_XQ + 2 * LQ].rearrange("p (t l) -> p t l", t=2)
            xk_sb = blob[:, C_XK : C_XK + 2 * LK].rearrange("p (t l) -> p t l", t=2)
            xv_sb = blob[:, C_XV : C_XV + 2 * LK].rearrange("p (t l) -> p t l", t=2)
            wq_sb = blob[:, C_W + 0 * 2 * D : C_W + 1 * 2 * D].rearrange(
                "p (t j) -> p t j", t=2
            )
            wk_sb = blob[:, C_W + 1 * 2 * D : C_W + 2 * 2 * D].rearrange(
                "p (t j) -> p t j", t=2
            )
            wv_sb = blob[:, C_W + 2 * 2 * D : C_W + 3 * 2 * D].rearrange(
                "p (t j) -> p t j", t=2
            )
            wo_sb = blob[:, C_W + 3 * 2 * D : C_W + 4 * 2 * D].rearrange(
                "p (t j) -> p t j", t=2
            )
            small = blob[:, C_SMALL : C_SMALL + 6 * D].rearrange(
                "p (k d) -> p k d", k=6
            )
            bq_sb = small[0:1, 0, :]
            bk_sb = small[0:1, 1, :]
            bv_sb = small[0:1, 2, :]
            bo_sb = small[0:1, 3, :]
            lng_sb = small[:, 4, :]
            lnb_sb = small[:, 5, :]
            idn_sb = blob[:, C_IDN : C_IDN + P]

            ones_sb = singles.tile([1, 512], BF16, tag="ones")
            nc.vector.memset(ones_sb, 1.0)
            eps_sb = singles.tile([P, 1], F32, tag="eps")
            nc.vector.memset(eps_sb, LN_EPS)

            # ---- persistent activations ----------------------------------
            QT_sb = singles.tile([P, 2, LQ], BF16, tag="QT")
            KT_sb = singles.tile([P, 2, LK], BF16, tag="KT")
            vaug = [
                singles.tile([P, H * 64], BF16, tag=f"vaug{t}", name=f"vaug{t}")
                for t in range(N_JT)
            ]
            ctxTn = singles.tile([P, 2, LQ], BF16, tag="ctxTn")
            y_sb = singles.tile([P, N_QT, D], F32, tag="y")
            yo_sb = singles.tile([P, N_QT, D], BF16, tag="yo")
            mv_sb = singles.tile([P, N_QT, 2], F32, tag="mv")
            sd_sb = singles.tile([P, N_QT], F32, tag="sd")
            rstd_sb = singles.tile([P, N_QT], F32, tag="rstd")

            # ---- phase A: QKV projections --------------------------------
            # QT[j, t] = sum_d WqT[d, j] * qT[d, t] + bq[j]
            for jt in range(2):
                for qcc in range(2):
                    ps = mmps.tile([P, 512], F32, tag="mm")
                    nc.tensor.matmul(
                        ps,
                        lhsT=wq_sb[:, 0, jt * P : (jt + 1) * P],
                        rhs=xq_sb[:, 0, qcc * 512 : (qcc + 1) * 512],
                        start=True,
                        stop=False,
                    )
                    nc.tensor.matmul(
                        ps,
                        lhsT=wq_sb[:, 1, jt * P : (jt + 1) * P],
                        rhs=xq_sb[:, 1, qcc * 512 : (qcc + 1) * 512],
                        start=False,
                        stop=False,
                    )
                    nc.tensor.matmul(
                        ps,
                        lhsT=bq_sb[:, jt * P : (jt + 1) * P],
                        rhs=ones_sb[:, :512],
                        start=False,
                        stop=True,
                    )
                    nc.vector.tensor_copy(
                        out=QT_sb[:, jt, qcc * 512 : (qcc + 1) * 512], in_=ps
                    )
            for jt in range(2):
                for kc in range(4):
                    ps = mmps.tile([P, 512], F32, tag="mm")
                    nc.tensor.matmul(
                        ps,
                        lhsT=wk_sb[:, 0, jt * P : (jt + 1) * P],
                        rhs=xk_sb[:, 0, kc * 512 : (kc + 1) * 512],
                        start=True,
                        stop=False,
                    )
                    nc.tensor.matmul(
                        ps,
                        lhsT=wk_sb[:, 1, jt * P : (jt + 1) * P],
                        rhs=xk_sb[:, 1, kc * 512 : (kc + 1) * 512],
                        start=False,
                        stop=False,
                    )
                    nc.tensor.matmul(
                        ps,
                        lhsT=bk_sb[:, jt * P : (jt + 1) * P],
                        rhs=ones_sb[:, :512],
                        start=False,
                        stop=True,
                    )
                    nc.vector.tensor_copy(
                        out=KT_sb[:, jt, kc * 512 : (kc + 1) * 512], in_=ps
                    )
            # V token-major, written interleaved into vaug with ones blocks
            for tt in range(N_JT):
                ps = mmps.tile([P, D], F32, tag="mm")
                nc.tensor.matmul(
                    ps,
                    lhsT=xv_sb[:, 0, tt * P : (tt + 1) * P],
                    rhs=wv_sb[:, 0, :],
                    start=True,
                    stop=False,
                )
                nc.tensor.matmul(
                    ps,
                    lhsT=xv_sb[:, 1, tt * P : (tt + 1) * P],
                    rhs=wv_sb[:, 1, :],
                    start=False,
                    stop=False,
                )
                nc.tensor.matmul(
                    ps,
                    lhsT=ones_sb[:1, :P],
                    rhs=bv_sb,
                    start=False,
                    stop=True,
                )
                vt = vaug[tt].rearrange("p (h c) -> p h c", c=64)
                nc.vector.memset(vt[:, :, DH:], 1.0)
                nc.vector.tensor_copy(
                    out=vt[:, :, :DH],
                    in_=ps.rearrange("p (h c) -> p h c", c=DH),
                )

            # ---- attention ----------------------------------------------
            for qc in range(N_QC):
                q0 = qc * 512
                cu = temps.tile([P, 2, 512], F32, tag="cu")  # unnormalized ctxT
                den = temps.tile([P, 2, 512], F32, tag="den")  # denominators
                for hp in range(4):  # head pairs (2hp, 2hp+1)
                    pv = pvps.tile([P, 2, 512], F32, tag="pv")
                    for jt in range(N_JT):
                        s = sps.tile([P, 2, 512], F32, tag="s")
                        for e in range(2):
                            h = 2 * hp + e
                            dt = h // 4
                            r0 = (h % 4) * DH
                            nc.tensor.matmul(
                                s[:, e, :],
                                lhsT=KT_sb[r0 : r0 + DH, dt, jt * P : (jt + 1) * P],
                                rhs=QT_sb[r0 : r0 + DH, dt, q0 : q0 + 512],
                                start=True,
                                stop=True,
                                tile_position=(r0, 0),
                            )
                        es = temps.tile([P, 2, 512], BF16, tag="es")
                        nc.scalar.activation(
                            out=es,
                            in_=s,
                            func=mybir.ActivationFunctionType.Exp,
                            scale=SCALE,
                        )
                        for e in range(2):
                            h = 2 * hp + e
                            # each head accumulates in its own PSUM bank
                            # (col-strip packing miscomputes on this stack)
                            nc.tensor.matmul(
                                pv[0:64, e, :],
                                lhsT=vaug[jt][:, 64 * h : 64 * h + 64],
                                rhs=es[:, e, :],
                                start=(jt == 0),
                                stop=(jt == N_JT - 1),
                            )
                    # stage ctx + denominator rows into SBUF at the ctxTn row
                    # layout (rows 32*(h%4) of partition-tile h//4); the
                    # reciprocal runs batched from SBUF afterwards (reciprocal
                    # with a PSUM source miscomputes/crashes on this stack)
                    for e in range(2):
                        h = 2 * hp + e
                        dt = h // 4
                        r0 = (h % 4) * DH
                        nc.vector.tensor_copy(
                            out=cu[r0 : r0 + DH, dt, :], in_=pv[0:DH, e, :]
                        )
                        nc.vector.tensor_copy(
                            out=den[r0 : r0 + DH, dt, :], in_=pv[DH:64, e, :]
                        )
                # normalize all 8 heads for this q chunk: 2 reciprocals + 2 mults
                rec = temps.tile([P, 2, 512], F32, tag="rec")
                nc.vector.reciprocal(out=rec, in_=den)
                for dtv in range(2):
                    nc.vector.tensor_tensor(
                        out=ctxTn[:, dtv, q0 : q0 + 512],
                        in0=cu[:, dtv, :],
                        in1=rec[:, dtv, :],
                        op=mybir.AluOpType.mult,
                    )

                # ---- output projection + residual for this q chunk -------
                for q4 in range(4):
                    qt = qc * 4 + q4
                    po = mmps.tile([P, D], F32, tag="mm")
                    nc.tensor.matmul(
                        po,
                        lhsT=ctxTn[:, 0, qt * P : (qt + 1) * P],
                        rhs=wo_sb[:, 0, :],
                        start=True,
                        stop=False,
                    )
                    nc.tensor.matmul(
                        po,
                        lhsT=ctxTn[:, 1, qt * P : (qt + 1) * P],
                        rhs=wo_sb[:, 1, :],
                        start=False,
                        stop=False,
                    )
                    nc.tensor.matmul(
                        po,
                        lhsT=ones_sb[:1, :P],
                        rhs=bo_sb,
                        start=False,
                        stop=False,
                    )
                    # residual: accumulate token-major q into the same PSUM
                    # tile via identity matmul: po[t, dt*128+n] += xq[n, t]
                    for dt in range(2):
                        nc.tensor.matmul(
                            po[:, dt * P : (dt + 1) * P],
                            lhsT=xq_sb[:, dt, qt * P : (qt + 1) * P],
                            rhs=idn_sb,
                            start=False,
                            stop=(dt == 1),
                        )
                    nc.vector.tensor_copy(out=y_sb[:, qt, :], in_=po)
                    st = temps.tile([P, 6], F32, tag="st")
                    nc.vector.bn_stats(out=st, in_=y_sb[:, qt, :])
                    nc.vector.bn_aggr(out=mv_sb[:, qt, :], in_=st)

            # ---- final LayerNorm pass (one ACT table switch) -------------
            nc.scalar.activation(
                out=sd_sb,
                in_=mv_sb[:, :, 1:2],
                func=mybir.ActivationFunctionType.Sqrt,
                bias=eps_sb,
            )
            nc.vector.reciprocal(out=rstd_sb, in_=sd_sb)
            for qt in range(N_QT):
                nc.vector.tensor_scalar(
                    out=y_sb[:, qt, :],
                    in0=y_sb[:, qt, :],
                    scalar1=mv_sb[:, qt, 0:1],
                    scalar2=rstd_sb[:, qt : qt + 1],
                    op0=mybir.AluOpType.subtract,
                    op1=mybir.AluOpType.mult,
                )
                nc.vector.tensor_tensor(
                    out=y_sb[:, qt, :],
                    in0=y_sb[:, qt, :],
                    in1=lng_sb,
                    op=mybir.AluOpType.mult,
                )
                nc.vector.tensor_add(
                    out=yo_sb[:, qt, :], in0=y_sb[:, qt, :], in1=lnb_sb
                )
            nc.sync.dma_start(
                out=out_d.rearrange("(t p) d -> p t d", p=P), in_=yo_sb
            )

    nc.finalize()
    return nc


_NC_CACHE = None


def _get_nc():
    global _NC_CACHE
    if _NC_CACHE is None:
        _NC_CACHE = build_nc()
    return _NC_CACHE


def _pack_blob(qT, kT, vT, WqT, WkT, WvT, WoT, smallv):
    """Pack per-core tensors into the [P, CBLOB] bf16 blob (tile layouts)."""
    blob = np.empty((P, CBLOB), dtype=NPBF16)

    def chan_major(x, cols):  # x [256, L] -> [128, 2*L] as (p, t, l)
        L = x.shape[1]
        blob[:, cols : cols + 2 * L] = (
            x.reshape(2, P, L).transpose(1, 0, 2).reshape(P, 2 * L)
        )

    chan_major(qT, C_XQ)
    chan_major(kT, C_XK)
    chan_major(vT, C_XV)
    for i, w in enumerate((WqT, WkT, WvT, WoT)):
        chan_major(w, C_W + i * 2 * D)
    # SMALL: 6 rows of 256, replicated across the 128 partitions
    blob[:, C_SMALL : C_SMALL + 6 * D] = smallv.reshape(1, 6 * D)
    blob[:, C_IDN : C_IDN + P] = np.eye(P, dtype=NPBF16)
    return blob


def make_in_maps(query, key, value, Wq, bq, Wk, bk, Wv, bv, Wo, bo, ln_g, ln_b):
    f16 = lambda x: np.asarray(x, dtype=np.float32).astype(NPBF16)
    WqT, WkT, WvT, WoT = (
        f16(np.asarray(w, dtype=np.float32).T) for w in (Wq, Wk, Wv, Wo)
    )
    smallv = f16(np.stack([bq, bk, bv, bo, ln_g, ln_b]))
    query = np.asarray(query, dtype=np.float32)
    key = np.asarray(key, dtype=np.float32)
    value = np.asarray(value, dtype=np.float32)
    in_maps = []
    for c in range(8):
        b, half = c // 2, c % 2
        lo = half * LQ
        blob = _pack_blob(
            f16(query[b, lo : lo + LQ, :].T),
            f16(key[b].T),
            f16(value[b].T),
            WqT,
            WkT,
            WvT,
            WoT,
            smallv,
        )
        in_maps.append({"blob": blob})
    return in_maps


def kernel(query, key, value, Wq, bq, Wk, bk, Wv, bv, Wo, bo, ln_g, ln_b):
    nc = _get_nc()
    in_maps = make_in_maps(
        query, key, value, Wq, bq, Wk, bk, Wv, bv, Wo, bo, ln_g, ln_b
    )
    res = run_bass_kernel_spmd(nc, in_maps, core_ids=list(range(8)))
    out = np.empty((4, 2048, 256), dtype=np.float32)
    for c in range(8):
        b, half = c // 2, c % 2
        out[b, half * LQ : (half + 1) * LQ, :] = np.asarray(
            res.results[c]["out"], dtype=np.float32
        )
    return out
